# revision 1
# baseline (speedup 1.0000x reference)
"""EuclidConv + training-mode BatchNorm on 8 Trainium2 NeuronCores.

Math (reference): out = BN(2*conv(x,w) + conv(x^2, ones3x3) + ||w_f||^2),
BN over global batch stats. The per-filter ||w||^2 term is channel-constant,
so BN's mean subtraction cancels it exactly -> never computed.

Sharding: OUTPUT-CHANNEL sharded (32 of 256 channels per core, all 32
images). This makes the BN statistics entirely core-local (collectives in
this environment cost ~40us flat, so batch-sharding loses). Images pack
4-at-a-time into the 128x128 PE array via tile_position col-tiling:
psum partition p = 32*j + c for image-slot j, channel c. fp16 operands
(11-bit mantissa ~ f32r precision at half the bytes, full PE rate).

Per 4-image block b (image slots j=0..3):
  u_j = x_j^2                                      (DVE / ACT, fp16)
  r4 psum[32j..32j+32] = ones32.T @ u_j            (channel sums, replicated
                                                    over the 32 channel rows)
  rc = r4 - 128*validmap                           (fp16, centered for
                                                    precision; one DVE op)
  box filter: vv = 3-tap vertical adds, t1f = 3-tap horizontal adds (DVE)
  conv psum accumulation group (one [128,1024] 2-bank tile, yt halves):
    identity.T @ t1f_view   (start=True: adds t1, clears psum)
    sum_k (2w)_k.T @ x_view (9 offsets x 4 col-tiles, fp16)
    ones1.T @ countmap_view (stop=True: re-adds 128*count, undoing centering)
  drain: ACT copy psum->s_sb with accum S; ACT square with accum Q
Stats: fold the 4 image-slots per channel with an fp32 mask matmul -> [32,2],
A = gamma*rsqrt(var+eps), B = beta - mean*A, broadcast back via fp32 matmul,
normalize out = s*A+B, DMA out.

Host-side prep is layout/sharding only: pad+transpose+cast of x, weight
transpose/scale, constant masks.
"""
import json

import numpy as np

import concourse.bass as bass
import concourse.mybir as mybir
import concourse.tile as tile
from concourse.ap import AP
from concourse.bass_utils import run_bass_kernel_spmd
from concourse.vector_clock import ScopedClock, VectorClock

F16 = mybir.dt.float16
F32 = mybir.dt.float32

N_CORES = 8
NIMG = 32
NBLK = 8
HP = 30
NPIX = HP * HP
NV = 28 * 28
YT_ROWS = 14
YT = YT_ROWS * 28
NHW = NIMG * NV
EPS = 1e-5

_split_ctr = [0]


def _split_waits_json(bir: bytes, max_waits: int = 1) -> bytes:
    """This container's walrus rejects instructions with >1 sync wait.
    Hoist excess waits onto EventSemaphore instructions inserted before the
    offender on the same engine stream."""
    m = json.loads(bir)
    for f in m["functions"]:
        for bb in f["blocks"]:
            newinsts = []
            for ins in bb["instructions"]:
                si = ins.get("sync_info")
                if si:
                    waits = si.get("on_wait") or []
                    if len(waits) > max_waits:
                        extra, keep = waits[:-max_waits], waits[-max_waits:]
                        for w_ in extra:
                            _split_ctr[0] += 1
                            newinsts.append(
                                {
                                    "debug": ins.get("debug", 0),
                                    "engine": ins["engine"],
                                    "ins": [],
                                    "outs": [],
                                    "name": f"antsplitw-{_split_ctr[0]}",
                                    "opcode": "EventSemaphore",
                                    "sync_info": {"on_update": [], "on_wait": [w_]},
                                }
                            )
                        si["on_wait"] = keep
                newinsts.append(ins)
            bb["instructions"] = newinsts
    return json.dumps(m).encode()


class _PatchedBass(bass.Bass):
    def to_json_bytes(self):
        return _split_waits_json(super().to_json_bytes())


class _SplitDrainTileContext(tile.TileContext):
    """Split the tile-exit drain's waits into single-wait drains (same
    walrus limitation as above)."""

    def _drain_and_barrier(self, tick_clock, wait_clock):
        g = tick_clock.global_clock
        n = len(g)
        for i in range(n):
            if g[i] > 0:
                vec = [0] * n
                vec[i] = g[i]
                d = self.nc.sync.drain()
                wait_clock.add_sem_waits(d.ins, ScopedClock({None: VectorClock(vec)}))
        self.nc.sync.drain()
        self.nc.all_engine_barrier()
        assert self.sems is not None
        popped = self.nc._tile_sem_poison_stack.pop()
        assert popped is self._sem_poison
        self.nc.clear_and_free_semaphores(list(self.sems.allocated().values()))
        self.nc.all_engine_barrier()


def _build_nc():
    nc = _PatchedBass()
    xh = nc.dram_tensor("xh", [128, NIMG * NPIX], F16, kind="ExternalInput")
    wt = nc.dram_tensor("wt", [128, 9 * 32], F16, kind="ExternalInput")
    cst16 = nc.dram_tensor("cst16", [128, 32], F16, kind="ExternalInput")
    comp16 = nc.dram_tensor("comp16", [128, 904], F16, kind="ExternalInput")
    cmap16 = nc.dram_tensor("cmap16", [1, 840], F16, kind="ExternalInput")
    onesr = nc.dram_tensor("onesr", [1, 128], F16, kind="ExternalInput")
    id128 = nc.dram_tensor("id128", [128, 128], F16, kind="ExternalInput")
    cst32 = nc.dram_tensor("cst32", [128, 40], F32, kind="ExternalInput")
    bc4 = nc.dram_tensor("bc4", [32, 128], F32, kind="ExternalInput")
    y = nc.dram_tensor("y", [NIMG, 32, 28, 28], F32, kind="ExternalOutput")

    with _SplitDrainTileContext(nc) as tc:
        with (
            tc.tile_pool(name="const", bufs=1) as cpool,
            tc.tile_pool(name="xpool", bufs=1) as xpool,
            tc.tile_pool(name="upool", bufs=1) as upool,
            tc.tile_pool(name="rpool", bufs=3) as rpool,
            tc.tile_pool(name="spool", bufs=1) as spool,
            tc.tile_pool(name="opool", bufs=3) as opool,
            tc.tile_pool(name="psc", bufs=2, space="PSUM") as psc,
            tc.tile_pool(name="psr", bufs=2, space="PSUM") as psr,
        ):
            wtile = cpool.tile([128, 9 * 32], F16, name="wtile")
            nc.sync.dma_start(wtile[:], wt[:])
            c16 = cpool.tile([128, 32], F16, name="c16")
            nc.gpsimd.dma_start(c16[:], cst16[:])
            compt = cpool.tile([128, 904], F16, name="compt")
            nc.sync.dma_start(compt[:], comp16[:])
            cmapt = cpool.tile([1, 840], F16, name="cmapt")
            nc.gpsimd.dma_start(cmapt[:], cmap16[:])
            onert = cpool.tile([1, 128], F16, name="onert")
            nc.sync.dma_start(onert[:], onesr[:])
            idt = cpool.tile([128, 128], F16, name="idt")
            nc.gpsimd.dma_start(idt[:], id128[:])
            c32 = cpool.tile([128, 40], F32, name="c32")
            nc.sync.dma_start(c32[:], cst32[:])
            bct = cpool.tile([32, 128], F32, name="bct")
            nc.gpsimd.dma_start(bct[:], bc4[:])
            ones32 = c16[0:128, 0:32]
            mask4 = c32[:, 0:32]

            s_sb = spool.tile([128, 16 * YT], F32, name="s_sb")
            sums = spool.tile([128, 8], F32, name="sums")
            sumsq = spool.tile([128, 8], F32, name="sumsq")

            xall = xpool.tile([128, NIMG * NPIX], F16, name="xall")
            for h in range(NIMG):
                eng = nc.sync if h % 2 == 0 else nc.gpsimd
                eng.dma_start(
                    xall[:, h * NPIX : (h + 1) * NPIX], xh[:, h * NPIX : (h + 1) * NPIX]
                )

            uts = []
            for b in range(NBLK):
                ut = upool.tile([128, 4 * NPIX], F16, name=f"ut{b}", tag=f"ut{b % 4}")
                sl = slice(b * 4 * NPIX, (b + 1) * 4 * NPIX)
                if b % 2 == 0:
                    nc.vector.tensor_mul(ut[:], xall[:, sl], xall[:, sl])
                else:
                    nc.scalar.activation(
                        ut[:], xall[:, sl], mybir.ActivationFunctionType.Square
                    )
                uts.append(ut)

            for b in range(NBLK):
                ut = uts[b]
                r4 = psr.tile([128, 904], F32, name=f"r4_{b}", tag="r4")
                for j in range(4):
                    for lo, hi in ((0, 512), (512, 900)):
                        nc.tensor.matmul(
                            r4[32 * j : 32 * j + 32, lo:hi],
                            ones32,
                            ut[:, j * NPIX + lo : j * NPIX + hi],
                            start=True,
                            stop=True,
                            tile_position=(0, 32 * j),
                            skip_group_check=True,
                        )
                rc16 = rpool.tile([128, 904], F16, name=f"rc16_{b}", tag="rc16")
                nc.vector.tensor_sub(rc16[:, 0:900], r4[:, 0:900], compt[:, 0:900])
                vv = rpool.tile([128, 840], F16, name=f"vv{b}", tag="vv")
                nc.vector.tensor_add(vv[:], rc16[:, 0:840], rc16[:, 30:870])
                nc.vector.tensor_add(vv[:], vv[:], rc16[:, 60:900])
                t1f = rpool.tile([128, 840], F16, name=f"t1f{b}", tag="t1f")
                nc.vector.tensor_add(t1f[:, 0:838], vv[:, 0:838], vv[:, 1:839])
                nc.vector.tensor_add(t1f[:, 0:838], t1f[:, 0:838], vv[:, 2:840])

                x3 = xall[:].rearrange("p (n a b) -> p n a b", a=HP, b=HP)
                ps = psc.tile([128, 1024], F32, name=f"ps{b}", tag="ps")
                t13 = t1f[:].rearrange("p (a c) -> p a c", c=HP)
                for yt in range(2):
                    y0 = yt * YT_ROWS
                    nc.tensor.matmul(
                        ps[:, 512 * yt : 512 * yt + YT],
                        idt[:],
                        t13[:, y0 : y0 + YT_ROWS, 0:28],
                        start=True,
                        stop=False,
                        skip_group_check=True,
                    )
                for k in range(9):
                    dy, dx = divmod(k, 3)
                    for j in range(4):
                        for yt in range(2):
                            y0 = yt * YT_ROWS
                            nc.tensor.matmul(
                                ps[32 * j : 32 * j + 32, 512 * yt : 512 * yt + YT],
                                wtile[:, k * 32 : (k + 1) * 32],
                                x3[:, b * 4 + j, y0 + dy : y0 + dy + YT_ROWS, dx : dx + 28],
                                start=False,
                                stop=False,
                                tile_position=(0, 32 * j),
                                skip_group_check=True,
                            )
                cm3 = cmapt[:].rearrange("p (a c) -> p a c", c=HP)
                for yt in range(2):
                    y0 = yt * YT_ROWS
                    nc.tensor.matmul(
                        ps[:, 512 * yt : 512 * yt + YT],
                        onert[:],
                        cm3[:, y0 : y0 + YT_ROWS, 0:28],
                        start=False,
                        stop=True,
                        skip_group_check=True,
                    )
                blk = b * 2 * YT
                psv = AP(ps.tensor, ps.offset, [[1024, 128], [512, 2], [1, YT]])
                nc.scalar.activation(
                    s_sb[:, blk : blk + 2 * YT],
                    psv,
                    mybir.ActivationFunctionType.Copy,
                    accum_out=sums[:, b : b + 1],
                )
                sq_scr = opool.tile([128, 2 * YT], F32, name="sq_scr", tag="sq")
                nc.scalar.activation(
                    sq_scr[:],
                    s_sb[:, blk : blk + 2 * YT],
                    mybir.ActivationFunctionType.Square,
                    accum_out=sumsq[:, b : b + 1],
                )

            sq2 = spool.tile([128, 2], F32, name="sq2")
            nc.vector.tensor_reduce(
                out=sq2[:, 0:1], in_=sums[:], op=mybir.AluOpType.add,
                axis=mybir.AxisListType.X,
            )
            nc.vector.tensor_reduce(
                out=sq2[:, 1:2], in_=sumsq[:], op=mybir.AluOpType.add,
                axis=mybir.AxisListType.X,
            )
            gstat = psr.tile([32, 2], F32, name="gstat", tag="r4")
            nc.tensor.matmul(gstat[:], mask4, sq2[:], start=True, stop=True)
            ab = spool.tile([32, 8], F32, name="ab")
            mean = ab[:, 0:1]
            qn = ab[:, 1:2]
            nc.vector.tensor_scalar_mul(mean, gstat[:, 0:1], 1.0 / NHW)
            nc.vector.tensor_scalar_mul(qn, gstat[:, 1:2], 1.0 / NHW)
            var = ab[:, 2:3]
            nc.vector.scalar_tensor_tensor(
                var, mean, 1.0, mean, op0=mybir.AluOpType.mult, op1=mybir.AluOpType.mult
            )
            nc.vector.tensor_sub(var, qn, var)
            sd = ab[:, 3:4]
            nc.scalar.activation(
                sd, var, mybir.ActivationFunctionType.Sqrt, bias=c32[0:32, 34:35]
            )
            abv = spool.tile([32, 2], F32, name="abv")
            nc.vector.reciprocal(abv[:, 0:1], sd)
            A = abv[:, 0:1]
            B = abv[:, 1:2]
            nc.vector.tensor_mul(A, A, c32[0:32, 32:33])
            nc.vector.scalar_tensor_tensor(
                B, mean, 1.0, A, op0=mybir.AluOpType.mult, op1=mybir.AluOpType.mult
            )
            nc.vector.tensor_sub(B, c32[0:32, 33:34], B)
            ab128p = psr.tile([128, 2], F32, name="ab128p", tag="r4")
            nc.tensor.matmul(ab128p[:], bct[:], abv[:], start=True, stop=True)
            ab128 = spool.tile([128, 2], F32, name="ab128")
            nc.vector.tensor_copy(ab128[:], ab128p[:])

            for b in range(NBLK):
                blk = b * 2 * YT
                o = opool.tile([128, 2 * YT], F32, name=f"o{b}", tag="o")
                if b % 3 == 0:
                    nc.vector.tensor_scalar(
                        o[:],
                        s_sb[:, blk : blk + 2 * YT],
                        ab128[:, 0:1],
                        ab128[:, 1:2],
                        op0=mybir.AluOpType.mult,
                        op1=mybir.AluOpType.add,
                    )
                elif b % 3 == 1:
                    nc.scalar.activation(
                        o[:],
                        s_sb[:, blk : blk + 2 * YT],
                        mybir.ActivationFunctionType.Identity,
                        bias=ab128[:, 1:2],
                        scale=ab128[:, 0:1],
                    )
                else:
                    nc.gpsimd.tensor_scalar(
                        o[:],
                        s_sb[:, blk : blk + 2 * YT],
                        ab128[:, 0:1],
                        ab128[:, 1:2],
                        op0=mybir.AluOpType.mult,
                        op1=mybir.AluOpType.add,
                    )
                for h in range(2):
                    dstap = AP(
                        y.ap().tensor,
                        b * 4 * 32 * NV + h * YT,
                        [[32 * NV, 4], [NV, 32], [1, YT]],
                    )
                    eng = nc.sync if (b + h) % 2 == 0 else nc.gpsimd
                    eng.dma_start(dstap, o[:, h * YT : (h + 1) * YT])
    return nc


def _prep_inputs(x, w, gamma, beta):
    x = np.asarray(x, np.float32)
    w = np.asarray(w, np.float32)
    gamma = np.asarray(gamma, np.float32)
    beta = np.asarray(beta, np.float32)

    xp = np.zeros((NIMG, 128, HP, HP), np.float32)
    xp[:, :, 1:29, 1:29] = x
    xh = np.ascontiguousarray(xp.transpose(1, 0, 2, 3)).reshape(128, NIMG * NPIX)
    xh = xh.astype(np.float16)

    cst16 = np.ones((128, 32), np.float16)

    comp16 = np.zeros((128, 904), np.float16)
    pidx = np.arange(900)
    yp, xpx = pidx // HP, pidx % HP
    valid = (yp >= 1) & (yp <= 28) & (xpx >= 1) & (xpx <= 28)
    comp16[:, :900] = (128.0 * valid)[None, :]

    jj = np.arange(840)
    jy, jx = jj // HP, jj % HP
    cy = np.where((jy == 0) | (jy == 27), 2, 3)
    cx = np.where((jx == 0) | (jx == 27), 2, 3)
    used = (jy < 28) & (jx < 28)
    cmap16 = np.where(used, 128.0 * cy * cx, 0.0).astype(np.float16)[None, :]
    onesr = np.ones((1, 128), np.float16)
    id128 = np.eye(128, dtype=np.float16)

    bc4 = np.zeros((32, 128), np.float32)
    for c in range(32):
        bc4[c, c::32] = 1.0
    mask4 = bc4.T.copy()

    maps = []
    for core in range(N_CORES):
        wtc = (2.0 * w[core * 32 : (core + 1) * 32]).reshape(32, 128, 9)
        wtc = np.ascontiguousarray(wtc.transpose(1, 2, 0)).reshape(128, 9 * 32)
        cst32 = np.zeros((128, 40), np.float32)
        cst32[:, 0:32] = mask4
        cst32[0:32, 32] = gamma[core * 32 : (core + 1) * 32]
        cst32[0:32, 33] = beta[core * 32 : (core + 1) * 32]
        cst32[0:32, 34] = EPS
        maps.append(
            {
                "xh": xh,
                "wt": wtc.astype(np.float16),
                "cst16": cst16,
                "comp16": comp16,
                "cmap16": cmap16,
                "onesr": onesr,
                "id128": id128,
                "cst32": cst32,
                "bc4": bc4,
            }
        )
    return maps


_NC_CACHE = []


def kernel(x, w, gamma, beta):
    if not _NC_CACHE:
        _NC_CACHE.append(_build_nc())
    nc = _NC_CACHE[0]
    maps = _prep_inputs(x, w, gamma, beta)
    res = run_bass_kernel_spmd(nc, maps, core_ids=list(range(N_CORES)))
    out = np.concatenate([r["y"] for r in res.results], axis=1)
    return np.ascontiguousarray(out.astype(np.float32))

